# revision 92
# baseline (speedup 1.0000x reference)
"""Trainium2 Bass kernel for the sparse (ragged) non-local attention block.

Math (per batch b, L = lengths[b], with q/k < N=4096, c < C=256, i < CI=128):
    theta = x @ theta_w + theta_b ; phi = x @ phi_w + phi_b ; g = x @ g_w + g_b
    s[q,k] = theta[q]·phi[k]   (k >= L masked to -inf)
    attn = softmax_k(s) ; y = attn @ g ; z = (y @ W_w + W_b + x) * (q < L)

Sharding: load-balanced query sharding within two 4-batch groups.
Batches are LPT-packed into two groups of 4 (cores 0-3 and 4-7). Each core
materializes phi/g for all four batches of its group in a static "key arena"
(bin j holds the core's j-th largest batch, padded to the static bin size
K_j = max over groups of the j-th largest ceil(L/128)). The query blocks of
each batch are dealt round-robin across the group's 4 cores into static
"slots"; every core runs the identical instruction stream (slot i always
attends over arena bin bin(i) with K_{bin(i)} key blocks) — which batch/query
rows live in a slot is decided purely by host-side input packing. Outputs are
slot-major; the host scatters them back to [B, N, C]. No collectives.

All raggedness is folded into per-core host-prepared inputs:
  - xt (key side, per arena bin): x[b]^T in bf16 with columns k >= L zeroed
    and bin-padding zeroed. Invalid/padding keys then have phi/g columns
    exactly 0 (phi_b is skipped on-chip: it shifts every valid key's score by
    a per-query constant, which softmax cancels), so they score s=0,
    p=exp(0)=1, and contribute p*g=0 to y.
  - ninv[slot] = -(K_bin*128 - L): corrects the denominator for those
    exp(0)=1 terms.
  - xr (slot-major) = (x rows + W_b + g_b @ W_w) * rowmask.
  - qm[slot]: per-row validity mask folded into the reciprocal.

On-chip per core:
  Projections run in fp8e4 DoubleRow (contraction 256 = both channel halves
  in one matmul): x and the theta/phi/g weights are fp8, weights pre-scaled
  x64 on the host (e4m3 would denormalize the raw 0.02-scale weights); the
  resulting 4096x score scale is removed for free by exp's scale parameter,
  and the 64x scale on y by dividing W_w by 64 host-side. Attention matmuls
  stay bf16 with f32 PSUM accumulation.
  thetaT [ci, slot*128] from xq; phiT/g over the arena (all three projection
  streams demand-paged into the attention loop so only bin 0's startup cost
  is exposed); then per slot chunk (up to 4 same-bin slots, 512 wide): for
  each 128-key block of the bin s^T[k,q] = phiT_kb^T @ thetaT (PE) ->
  p = exp(s/4096) (ScalarE, bf16) -> yT[ci,q] += g_kb^T @ p (PE, PSUM accum)
  and psb[lane] += p (DVE, bf16, one wide op per exp group); then fold psb
  lanes, per-slot denominator via psb-as-stationary matmul
  (psb_block.T @ ones -> [q-partition, 1]), r = qm/(denom+ninv);
  w = yT_qb^T @ W_w (PE, separate PSUM pool so the scores pipeline never
  blocks on finish-stage consumers), out = w * r + xr (one fused DVE op) ->
  DMA out (slot-major, whole chunk in one 4KB-line transfer).
"""

import sys

if "/opt/trn_rl_repo" not in sys.path:
    sys.path.insert(0, "/opt/trn_rl_repo")

import contextlib
import ctypes
import types

import ml_dtypes
import numpy as np

import concourse.bass as bass
import concourse.mybir as mybir
import concourse.tile as tile
from concourse import bacc
from concourse.bass import ts

B, N, C, CI = 8, 4096, 256, 128
P = 128
NKB = N // P  # 32 key blocks
QC = 512  # query chunk (4 slots)
GSZ = 2  # key blocks per exp group (2 PSUM banks wide)
CPG = 4  # cores per group

dt = mybir.dt
AF = mybir.ActivationFunctionType
OP = mybir.AluOpType

LAST_EXEC_NS = None


def _install_ntff_shim():
    """Register the axon NTFF profile hook (missing antenv.axon_hooks in this
    image) so run_bass_kernel_spmd(trace=True) can report HW exec time."""
    if "antenv.axon_hooks" in sys.modules:
        return
    try:
        import antenv

        mod = types.ModuleType("antenv.axon_hooks")
        _state = {"hook": None}
        mod.set_axon_ntff_profile_hook = lambda h: _state.__setitem__("hook", h)
        mod.get_axon_ntff_profile_hook = lambda: _state["hook"]
        sys.modules["antenv.axon_hooks"] = mod
        antenv.axon_hooks = mod

        lib = ctypes.CDLL("/opt/axon/libaxon_pjrt.so")
        if not hasattr(lib, "axon_start_nrt_profile"):
            return
        lib.axon_start_nrt_profile.argtypes = [
            ctypes.POINTER(ctypes.c_int64),
            ctypes.c_size_t,
        ]
        lib.axon_start_nrt_profile.restype = ctypes.c_int64
        lib.axon_stop_nrt_profile.argtypes = [ctypes.c_char_p]
        lib.axon_stop_nrt_profile.restype = ctypes.c_int64

        @contextlib.contextmanager
        def _hook(output_dir, device_ids):
            import jax

            jax.devices()
            if device_ids:
                ids = (ctypes.c_int64 * len(device_ids))(*device_ids)
                rc = lib.axon_start_nrt_profile(ids, len(device_ids))
            else:
                rc = lib.axon_start_nrt_profile(None, 0)
            if rc != 0:
                raise RuntimeError(f"axon_start_nrt_profile rc={rc}")
            try:
                yield
            finally:
                n = lib.axon_stop_nrt_profile(str(output_dir).encode())
                if n < 0:
                    raise RuntimeError(f"axon_stop_nrt_profile rc={n}")

        mod.set_axon_ntff_profile_hook(_hook)
    except Exception:
        pass


def build(K):
    # K = static per-bin key-block counts (elementwise max over the two
    # groups of the sorted-descending per-batch block counts).
    nbins = len(K)
    S_bins = [-(-k // CPG) for k in K]  # per-core slots per bin
    S = sum(S_bins)
    AR = sum(K)  # arena key blocks
    arena_off = []
    o = 0
    for k in K:
        arena_off.append(o)
        o += k

    # slot chunks: up to 4 same-bin slots processed together (512 queries)
    chunks = _chunks_of(K)  # (bin, slot0, nb)

    nc = bacc.Bacc("TRN2", target_bir_lowering=False, debug=False, num_devices=B)

    # xq/xt pack the two 128-channel halves of each piece adjacently so one
    # DMA moves 2KB+ contiguous per partition line (descriptor-dominated
    # otherwise); xr/out are [P, slot*C] so a whole chunk moves in one DMA.
    xq = nc.declare_dram_parameter("xq", [P, 2 * S * P], dt.float8e4, False)
    xt = nc.declare_dram_parameter("xt", [P, 2 * AR * P], dt.float8e4, False)
    xr = nc.declare_dram_parameter("xr", [P, S * C], dt.float32, False)
    wts = nc.declare_dram_parameter("wts", [P, 6 * CI], dt.float8e4, False)
    ww = nc.declare_dram_parameter("ww", [CI, C], dt.bfloat16, False)
    tb = nc.declare_dram_parameter("tb", [P, 1], dt.float32, False)
    qm2 = nc.declare_dram_parameter("qm2", [P, S], dt.float32, False)
    ninv = nc.declare_dram_parameter("ninv", [P, S], dt.float32, False)
    out = nc.declare_dram_parameter("out", [P, S * C], dt.float32, True)

    NQ = S * P  # total slot queries
    NK = AR * P  # total arena keys

    with tile.TileContext(nc) as tc:
        with (
            tc.tile_pool(name="wpool", bufs=1) as wpool,
            tc.tile_pool(name="xtp", bufs=1) as xtp,
            tc.tile_pool(name="feat", bufs=1) as feat,
            tc.tile_pool(name="ppool", bufs=8) as ppool,
            tc.tile_pool(name="psbp", bufs=4) as psbp,
            tc.tile_pool(name="ysbp", bufs=2) as ysbp,
            tc.tile_pool(name="smallp", bufs=2) as smallp,
            tc.tile_pool(name="xrp", bufs=8) as xrp,
            tc.tile_pool(name="outp", bufs=4) as outp,
            tc.tile_pool(name="sc_ps", bufs=2, space="PSUM") as sc_ps,
            tc.tile_pool(name="y_ps", bufs=2, space="PSUM") as y_ps,
            tc.tile_pool(name="fin_ps", bufs=2, space="PSUM") as fin_ps,
        ):
            # ---- constants / weights to SBUF ----
            # all projection weights in one contiguous transfer (1.5KB lines)
            wts_s = wpool.tile([P, 6 * CI], dt.float8e4, tag="wts")
            nc.sync.dma_start(wts_s[:], wts.ap()[:])
            tb_s = wpool.tile([P, 1], dt.float32, tag="tb")
            nc.sync.dma_start(tb_s[:], tb.ap()[:])
            ones_s = wpool.tile([P, 1], dt.bfloat16, tag="ones")
            nc.vector.memset(ones_s[:], 1.0)

            # query-side / key-side x tiles, loaded in pieces ordered by the
            # first attention chunk that needs each piece
            xq_s = xtp.tile([P, 2 * NQ], dt.float8e4, tag="xq")
            xt_s = xtp.tile([P, 2 * NK], dt.float8e4, tag="xt")
            kpieces = _kpieces_of(K)

            # DMA release discipline: sync-queue DMAs all dispatch at t=0 and
            # SDMA fair-shares bandwidth across everything pending, starving
            # the critical first pieces. So only the first pieces go on the
            # sync queue; the rest are issued from the Scalar (HWDGE) queue
            # at demand-paged program positions so they dispatch in pipeline
            # order, just ahead of their consumers.
            ww_s = wpool.tile([CI, C], dt.bfloat16, tag="ww")
            qm_s = wpool.tile([P, S], dt.float32, tag="qm")
            ninv_s = wpool.tile([P, S], dt.float32, tag="ninv")
            late_consts = [False]

            def load_late_consts():
                if not late_consts[0]:
                    nc.sync.dma_start(ww_s[:], ww.ap()[:])
                    nc.sync.dma_start(qm_s[:], qm2.ap()[:])
                    nc.sync.dma_start(ninv_s[:], ninv.ap()[:])
                    late_consts[0] = True

            xt_dma = [0]
            xq_dma = [0]

            def release_xt(upto):
                while xt_dma[0] < min(upto, len(kpieces)):
                    kc, w = kpieces[xt_dma[0]]
                    eng = nc.sync
                    eng.dma_start(
                        xt_s[:, 2 * kc : 2 * (kc + w)],
                        xt.ap()[:, 2 * kc : 2 * (kc + w)],
                    )
                    xt_dma[0] += 1

            def release_xq(upto):
                while xq_dma[0] < min(upto, len(chunks)):
                    _, sl0, nb = chunks[xq_dma[0]]
                    c0 = sl0 * P
                    w = nb * P
                    eng = nc.sync
                    eng.dma_start(
                        xq_s[:, 2 * c0 : 2 * (c0 + w)],
                        xq.ap()[:, 2 * c0 : 2 * (c0 + w)],
                    )
                    xq_dma[0] += 1

            release_xq(1)
            release_xt(3)

            # ---- projections (issued lazily, interleaved with attention) ----
            thetaT = feat.tile([P, NQ], dt.bfloat16, tag="thetaT")
            phiT = feat.tile([P, NK], dt.bfloat16, tag="phiT")
            g_s = feat.tile([P, NK], dt.bfloat16, tag="g")

            def proj_theta(ch):
                release_xq(ch + 1)
                _, sl0, nb = chunks[ch]
                w = nb * P
                c0 = sl0 * P
                pth = fin_ps.tile([P, QC], dt.float32, tag="fin", name="pth")
                nc.tensor.matmul(
                    pth[:, :w],
                    lhsT=wts_s[:, 0 : 2 * CI].rearrange("p (t c) -> p t c", t=2),
                    rhs=xq_s[:, 2 * c0 : 2 * c0 + 2 * w].rearrange(
                        "p (t c) -> p t c", t=2
                    ),
                    start=True,
                    stop=True,
                    perf_mode=mybir.MatmulPerfMode.DoubleRow,
                )
                nc.vector.tensor_scalar_add(
                    thetaT[:, c0 : c0 + w], pth[:, :w], tb_s[:, 0:1]
                )

            def proj_phi(kc, w):
                release_xt(kc // QC + 1)
                pph = fin_ps.tile([P, QC], dt.float32, tag="fin", name="pph")
                nc.tensor.matmul(
                    pph[:, :w],
                    lhsT=wts_s[:, 2 * CI : 4 * CI].rearrange(
                        "p (t c) -> p t c", t=2
                    ),
                    rhs=xt_s[:, 2 * kc : 2 * kc + 2 * w].rearrange(
                        "p (t c) -> p t c", t=2
                    ),
                    start=True,
                    stop=True,
                    perf_mode=mybir.MatmulPerfMode.DoubleRow,
                )
                nc.vector.tensor_copy(phiT[:, kc : kc + w], pph[:, :w])

            def proj_g(kbp):
                release_xt((2 * kbp + 1) // 4 + 1)
                pg = fin_ps.tile([P, QC], dt.float32, tag="fin", name="pg")
                nh = min(2, AR - kbp * 2)
                for h in range(nh):
                    kb = kbp * 2 + h
                    kc_p, w_p = kpieces[kb // 4]
                    d = (kb - kc_p // P) * P
                    nc.tensor.matmul(
                        pg[:, h * CI : (h + 1) * CI],
                        lhsT=xt_s[:, 2 * kc_p : 2 * kc_p + 2 * w_p].rearrange(
                            "p (t c) -> p t c", t=2
                        )[:, :, d : d + P],
                        rhs=wts_s[:, 4 * CI : 6 * CI].rearrange(
                            "p (t c) -> p t c", t=2
                        ),
                        start=True,
                        stop=True,
                        perf_mode=mybir.MatmulPerfMode.DoubleRow,
                    )
                eng = nc.scalar if kbp % 2 == 0 else nc.vector
                cp = eng.copy if kbp % 2 == 0 else eng.tensor_copy
                cp(
                    g_s[:, kbp * 2 * P : kbp * 2 * P + nh * P].rearrange(
                        "p (h c) -> p h c", h=nh
                    ),
                    pg[:, 0 : nh * CI].rearrange("p (h c) -> p h c", h=nh),
                )

            # demand-paged projection issuance: each phi/g piece is issued
            # just before its first consumer, with a fixed lookahead cadence
            # so later bins' projections hide inside earlier bins' attention.
            NGP = -(-AR // 2)  # g pairs
            theta_next = [0]
            phi_next = [0]
            g_next = [0]
            phi_done = [-1]  # highest arena block whose phi is issued
            g_done = [-1]

            def ensure_theta(ch):
                while theta_next[0] <= ch:
                    proj_theta(theta_next[0])
                    theta_next[0] += 1

            def issue_phi():
                i = phi_next[0]
                if i < len(kpieces):
                    kc, w = kpieces[i]
                    proj_phi(kc, w)
                    phi_next[0] += 1
                    phi_done[0] = (kc + w) // P - 1

            def ensure_phi(b):
                while phi_done[0] < b and phi_next[0] < len(kpieces):
                    issue_phi()

            def issue_g():
                i = g_next[0]
                if i < NGP:
                    proj_g(i)
                    g_next[0] += 1
                    g_done[0] = min(AR - 1, i * 2 + 1)

            def ensure_g(b):
                while g_done[0] < b and g_next[0] < NGP:
                    issue_g()

            gctr = [0]

            def step_proj():
                gctr[0] += 1
                load_late_consts()
                # stream input DMAs a few pieces ahead of the projections
                release_xt(max(phi_next[0], g_next[0] * 2 // 4) + 4)
                release_xq(theta_next[0] + 2)
                issue_g()
                if gctr[0] % 2 == 0:
                    issue_phi()
                if gctr[0] % 8 == 5 and theta_next[0] < len(chunks):
                    proj_theta(theta_next[0])
                    theta_next[0] += 1

            # ---- attention, software-pipelined one exp-group deep ----
            qstate = {}

            # key blocks per exp group: the 2-bank score tile holds GSZ*QC
            # f32 elements, so narrow chunks pack more key blocks per group
            # (w=128 -> 8), amortizing per-instruction overheads. The lane
            # stride must keep every matmul output within one PSUM bank:
            # stride w only if w divides the bank (512), else pad to 512.
            def lam_of(w):
                return w if (QC % w) == 0 else QC

            def gcap_of(w):
                return max(1, (GSZ * QC) // lam_of(w))

            def start_qc(qc):
                _, sl0, nb = chunks[qc]
                ysum = y_ps.tile([P, QC], dt.float32, tag="ysum", name="ysum")
                psb = psbp.tile([P, GSZ * QC], dt.bfloat16, tag="psb", name="psb")
                xr_t = xrp.tile([P, 4 * C], dt.float32, tag="xr", name="xr_t")
                nc.sync.dma_start(
                    xr_t[:, : nb * C], xr.ap()[:, sl0 * C : (sl0 + nb) * C]
                )
                qstate[qc] = (ysum, psb, xr_t)

            def drain(item):
                qc, kb0, gsz, p = item
                bin_j, sl0, nb = chunks[qc]
                kj = K[bin_j]
                ao = arena_off[bin_j]
                w = nb * P
                if kb0 == 0:
                    start_qc(qc)
                ysum, psb, xr_t = qstate[qc]
                ensure_g(ao + kb0 + gsz - 1)
                lam = lam_of(w)
                for j in range(gsz):
                    kbj = kb0 + j
                    nc.tensor.matmul(
                        ysum[:, :w],
                        lhsT=g_s[:, ts(ao + kbj, P)],
                        rhs=p[:, j * lam : j * lam + w],
                        start=(kbj == 0),
                        stop=(kbj == kj - 1),
                        skip_group_check=True,
                    )
                # one wide DVE op accumulates the whole exp-group into the
                # per-lane partial sums (lane l holds key blocks kb0+l)
                if lam == w:
                    pv = p[:, : gsz * w]
                    bv = psb[:, : gsz * w]
                else:
                    pv = p.rearrange("a (g q) -> a g q", g=GSZ)[:, :gsz, :w]
                    bv = psb.rearrange("a (g q) -> a g q", g=GSZ)[:, :gsz, :w]
                if kb0 == 0:
                    nc.vector.tensor_copy(bv, pv)
                else:
                    nc.vector.tensor_add(bv, bv, pv)
                if kb0 + gsz == kj:
                    finish_queue.append([qc, 0])

            def finish_qc(qc):
                bin_j, sl0, nb = chunks[qc]
                w = nb * P
                kj = K[bin_j]
                ysum, psb, xr_t = qstate.pop(qc)
                # free the ysum bank as early as possible
                y_sb = ysbp.tile([P, QC], dt.bfloat16, tag="ysb", name="y_sb")
                nc.vector.tensor_copy(y_sb[:, :w], ysum[:, :w])
                # tree-fold partial-sum lanes into lane 0, then per-slot key
                # reduction with psb as the stationary: lands [q-part, 1].
                lam = lam_of(w)
                lanes = min(kj, gcap_of(w))
                if lam != w:
                    for s in range(1, lanes):
                        nc.vector.tensor_add(
                            psb[:, :w], psb[:, :w], psb[:, s * lam : s * lam + w]
                        )
                    lanes = 1
                while lanes > 1:
                    if lanes % 2 == 1:
                        nc.vector.tensor_add(
                            psb[:, :w],
                            psb[:, :w],
                            psb[:, (lanes - 1) * w : lanes * w],
                        )
                        lanes -= 1
                    else:
                        half = lanes // 2
                        nc.vector.tensor_add(
                            psb[:, : half * w],
                            psb[:, : half * w],
                            psb[:, half * w : lanes * w],
                        )
                        lanes = half
                ds = fin_ps.tile([P, QC], dt.float32, tag="fin", name="ds")
                for j in range(nb):
                    nc.tensor.matmul(
                        ds[:, j : j + 1],
                        lhsT=psb[:, ts(j, P)],
                        rhs=ones_s[:, 0:1],
                        start=True,
                        stop=True,
                        skip_group_check=True,
                    )
                dn = smallp.tile([P, 4], dt.float32, tag="dn", name="dn")
                nc.vector.tensor_add(
                    dn[:, :nb], ds[:, 0:nb], ninv_s[:, sl0 : sl0 + nb]
                )
                rc = smallp.tile([P, 4], dt.float32, tag="rc", name="rc")
                nc.vector.reciprocal(rc[:, :nb], dn[:, :nb])
                r_t = smallp.tile([P, 4], dt.float32, tag="rt", name="r_t")
                nc.vector.tensor_mul(r_t[:, :nb], rc[:, :nb], qm_s[:, sl0 : sl0 + nb])
                ot = outp.tile([P, 4 * C], dt.float32, tag="ot", name="ot")
                for j in range(nb):
                    wy = fin_ps.tile([P, QC], dt.float32, tag="fin", name="wy")
                    nc.tensor.matmul(
                        wy[:, 0:C],
                        lhsT=y_sb[:, ts(j, P)],
                        rhs=ww_s[:],
                        start=True,
                        stop=True,
                    )
                    nc.vector.scalar_tensor_tensor(
                        ot[:, j * C : (j + 1) * C],
                        wy[:, 0:C],
                        r_t[:, j : j + 1],
                        xr_t[:, j * C : (j + 1) * C],
                        OP.mult,
                        OP.add,
                    )
                nc.sync.dma_start(
                    out.ap()[:, sl0 * C : (sl0 + nb) * C], ot[:, : nb * C]
                )

            pending = []
            finish_queue = []

            def tick_finishes(force=False):
                for ent in list(finish_queue):
                    ent[1] += 1
                    if force or ent[1] > 2:
                        finish_qc(ent[0])
                        finish_queue.remove(ent)

            for qc, (bin_j, sl0, nb) in enumerate(chunks):
                kj = K[bin_j]
                ao = arena_off[bin_j]
                w = nb * P
                c0 = sl0 * P
                ensure_theta(qc)
                gcap = gcap_of(w)
                groups = []
                _kb = 0
                while _kb < kj:
                    g = min(gcap, kj - _kb)
                    groups.append((_kb, g))
                    _kb += g
                lam = lam_of(w)
                for kb0, gsz in groups:
                    ensure_phi(ao + kb0 + gsz - 1)
                    sc = sc_ps.tile([P, GSZ * QC], dt.float32, tag="sc", name="sc")
                    for j in range(gsz):
                        nc.tensor.matmul(
                            sc[:, j * lam : j * lam + w],
                            lhsT=phiT[:, ts(ao + kb0 + j, P)],
                            rhs=thetaT[:, c0 : c0 + w],
                            start=True,
                            stop=True,
                        )
                    p = ppool.tile([P, GSZ * QC], dt.bfloat16, tag="p", name="p")
                    if lam == w:
                        pv = p[:, : gsz * w]
                        sv = sc[:, : gsz * w]
                    else:
                        pv = p.rearrange("a (g q) -> a g q", g=GSZ)[:, :gsz, :w]
                        sv = sc.rearrange("a (g q) -> a g q", g=GSZ)[:, :gsz, :w]
                    nc.scalar.activation(pv, sv, AF.Exp, scale=1.0 / 4096.0)
                    pending.append((qc, kb0, gsz, p))
                    step_proj()
                    if len(pending) > 2:
                        drain(pending.pop(0))
                        tick_finishes()
            while pending:
                drain(pending.pop(0))
                tick_finishes()
            tick_finishes(force=True)

    nc.compile()
    return nc


def _chunks_of(K):
    S_bins = [-(-k // CPG) for k in K]
    chunks = []
    s0 = 0
    for j in range(len(K)):
        left = S_bins[j]
        base = s0
        while left > 0:
            nb = min(QC // P, left)
            chunks.append((j, base, nb))
            base += nb
            left -= nb
        s0 += S_bins[j]
    return chunks


def _kpieces_of(K):
    NK = sum(K) * P
    kpieces = []
    kc = 0
    while kc < NK:
        w = min(QC, NK - kc)
        kpieces.append((kc, w))
        kc += w
    return kpieces


def _plan(lens):
    """LPT-pack the 8 batches into two groups of 4; derive the static bin
    sizes K and the per-core slot assignment."""
    nkbs = [max(1, -(-L // P)) if L > 0 else 1 for L in lens]
    order = sorted(range(B), key=lambda b: -nkbs[b])
    groups = [[], []]
    work = [0, 0]
    for b in order:
        # pick the lighter group that still has room
        cand = sorted(range(2), key=lambda g: (work[g], g))
        g = next(gg for gg in cand if len(groups[gg]) < CPG)
        groups[g].append(b)
        work[g] += nkbs[b] * nkbs[b]
    # bins: j-th largest batch of each group
    for g in range(2):
        groups[g].sort(key=lambda b: -nkbs[b])
    K = tuple(
        max(nkbs[groups[0][j]], nkbs[groups[1][j]]) for j in range(CPG)
    )
    S_bins = [-(-k // CPG) for k in K]
    S = sum(S_bins)
    # per-core slot tables: core = g*CPG + k; slot entries are
    # (batch, qblock) or None (dummy)
    slot_tables = []
    for g in range(2):
        for k in range(CPG):
            slots = []
            for j in range(CPG):
                b = groups[g][j]
                for si in range(S_bins[j]):
                    qb = si * CPG + k
                    slots.append((b, qb) if qb < nkbs[b] else None)
            slot_tables.append(slots)
    return nkbs, groups, K, S_bins, S, slot_tables


_NC_CACHE = {}


def kernel(**inputs):
    global LAST_EXEC_NS
    _install_ntff_shim()
    from concourse.bass_utils import run_bass_kernel_spmd

    x = np.asarray(inputs["x"], dtype=np.float32)
    lengths = np.asarray(inputs["lengths"]).astype(np.int64)
    theta_w = np.asarray(inputs["theta_w"], np.float32)
    theta_b = np.asarray(inputs["theta_b"], np.float32)
    phi_w = np.asarray(inputs["phi_w"], np.float32)
    g_w = np.asarray(inputs["g_w"], np.float32)
    g_b = np.asarray(inputs["g_b"], np.float32)
    W_w = np.asarray(inputs["W_w"], np.float32)
    W_b = np.asarray(inputs["W_b"], np.float32)

    bf16 = ml_dtypes.bfloat16
    f8 = ml_dtypes.float8_e4m3
    _t = theta_w.reshape(2, P, CI) * 64.0
    _p = phi_w.reshape(2, P, CI) * 64.0
    _g = g_w.reshape(2, P, CI) * 64.0
    wts_np = np.ascontiguousarray(
        np.concatenate([_t[0], _t[1], _p[0], _p[1], _g[0], _g[1]], axis=1)
    ).astype(f8)
    ww_np = np.ascontiguousarray(W_w / 64.0).astype(bf16)
    tb_np = np.ascontiguousarray(theta_b.reshape(P, 1) * 64.0).astype(np.float32)
    resid_base = (W_b + g_b @ W_w)[None, :].astype(np.float32)

    lens = [max(0, min(N, int(lengths[b]))) for b in range(B)]
    nkbs, groups, K, S_bins, S, slot_tables = _plan(lens)
    AR = sum(K)

    chunks = _chunks_of(K)
    kpieces = _kpieces_of(K)

    # per-group key arenas (shared by the 4 cores of a group), packed with
    # the two channel halves of each 512-col piece adjacent
    xt_groups = []
    for g in range(2):
        arena = np.zeros((2, P, AR * P), dtype=ml_dtypes.float8_e4m3)
        o = 0
        for j in range(CPG):
            b = groups[g][j]
            L = lens[b]
            keymask = (np.arange(N) < L).astype(np.float32)
            xz = (x[b] * keymask[:, None]).T  # [C, N]
            nkc = nkbs[b] * P
            arena[:, :, o * P : o * P + nkc] = (
                xz[:, :nkc].reshape(2, P, nkc).astype(ml_dtypes.float8_e4m3)
            )
            o += K[j]
        xt_g = np.zeros((P, 2 * AR * P), dtype=ml_dtypes.float8_e4m3)
        for kc, w in kpieces:
            xt_g[:, 2 * kc : 2 * kc + w] = arena[0, :, kc : kc + w]
            xt_g[:, 2 * kc + w : 2 * kc + 2 * w] = arena[1, :, kc : kc + w]
        xt_groups.append(xt_g)

    in_maps = []
    for core in range(B):
        g = core // CPG
        slots = slot_tables[core]
        xq_np = np.zeros((P, 2 * S * P), dtype=ml_dtypes.float8_e4m3)
        xr_np = np.zeros((P, S * C), dtype=np.float32)
        qm_np = np.zeros((P, S), dtype=np.float32)
        ninv_np = np.zeros((P, S), dtype=np.float32)
        bin_of = []
        for j in range(CPG):
            bin_of += [j] * S_bins[j]
        chunk_of_slot = {}
        for bj, sl0, nb in chunks:
            for jj in range(nb):
                chunk_of_slot[sl0 + jj] = (sl0, nb)
        for si, ent in enumerate(slots):
            if ent is None:
                continue
            b, qb = ent
            L = lens[b]
            rows = x[b, qb * P : (qb + 1) * P, :]  # [P, C]
            rt = rows.T.reshape(2, P, P).astype(ml_dtypes.float8_e4m3)
            sl0, nb = chunk_of_slot[si]
            w = nb * P
            c0 = sl0 * P
            d = (si - sl0) * P
            xq_np[:, 2 * c0 + d : 2 * c0 + d + P] = rt[0]
            xq_np[:, 2 * c0 + w + d : 2 * c0 + w + d + P] = rt[1]
            rmask = (np.arange(qb * P, (qb + 1) * P) < L).astype(np.float32)
            xr_np[:, si * C : (si + 1) * C] = (rows + resid_base) * rmask[:, None]
            qm_np[:, si] = rmask
            kproc = K[bin_of[si]] * P
            ninv_np[:, si] = -(kproc - L) + (1.0 if L == 0 else 0.0)
        in_maps.append(
            {
                "xq": xq_np,
                "xt": xt_groups[g],
                "xr": xr_np,
                "wts": wts_np,
                "ww": ww_np,
                "tb": tb_np,
                "qm2": qm_np,
                "ninv": ninv_np,
            }
        )

    if K not in _NC_CACHE:
        _NC_CACHE[K] = build(K)
    nc = _NC_CACHE[K]

    res = run_bass_kernel_spmd(nc, in_maps, list(range(B)))
    LAST_EXEC_NS = res.exec_time_ns

    out = np.zeros((B, N, C), dtype=np.float32)
    for core in range(B):
        o_core = np.asarray(res.results[core]["out"])  # [P, S*C]
        for si, ent in enumerate(slot_tables[core]):
            if ent is None:
                continue
            b, qb = ent
            out[b, qb * P : (qb + 1) * P, :] = o_core[:, si * C : (si + 1) * C]
    return out


if __name__ == "__main__":
    rng = np.random.default_rng(0)
    demo = {
        "x": rng.standard_normal((B, N, C), dtype=np.float32),
        "lengths": rng.integers(N // 2, N + 1, size=(B,)).astype(np.int32),
        "g_w": (rng.standard_normal((C, CI)) * 0.02).astype(np.float32),
        "g_b": np.zeros(CI, np.float32),
        "theta_w": (rng.standard_normal((C, CI)) * 0.02).astype(np.float32),
        "theta_b": np.zeros(CI, np.float32),
        "phi_w": (rng.standard_normal((C, CI)) * 0.02).astype(np.float32),
        "phi_b": np.zeros(CI, np.float32),
        "W_w": (rng.standard_normal((CI, C)) * 0.02).astype(np.float32),
        "W_b": np.zeros(C, np.float32),
    }
    o = kernel(**demo)
    print("out", o.shape, o.dtype, float(np.abs(o).mean()))
